# revision 26
# baseline (speedup 1.0000x reference)
"""Two-layer GCN (PyG GCNConv x2 + ReLU) on 8 Trainium2 NeuronCores.

Strategy (graph/data parallel, dst-partitioned):
  - Nodes are sharded across the 8 cores (12500 each); edges are partitioned
    by destination node so every scatter-add is core-local, accumulated in
    PSUM per 128-row output tile via matmuls against a selection matrix
    S[edge, row] = (dstrow[edge] == row) built with one is_equal per tile.
  - All normalization is folded into the data: layer-1 streams host
    pre-gathered, dinv-pre-scaled source rows (bf16) per tile -- a pure
    contiguous HWDGE DMA, no descriptor generation on the Q7.  The per-tile
    self-loop rows are chunk 0 of the same stream with dstrow = iota.
  - Layer-2 messages are gathered on-device from an AllGathered table of
    dinv-pre-scaled layer-1 outputs (bf16 rows padded to 256 B).  The 392
    InstDMAGatherAnt calls are striped across all 4 SWDGE queues so all 8
    Q7 cores generate descriptors concurrently (the ucode dedicates core
    pair 2q/2q+1 to queue q; a single queue leaves 6 cores idle).
  - The halo AllGather is split in two halves (table laid out as all cores'
    first half-shards then all second half-shards) and the bucket-2/3
    gathers are issued GATHER_DELAY tiles behind the bucket-0/1 gathers, so
    the second collective transfers underneath early gathers.
  - Per-node scales ride the Scalar (ACT) engine: t2 = relu(dinv^2*(agg@W1)
    + dinv*b1) and out = dinv*(agg@W2) + b2 are one activation instruction
    each, with the bias term added in PSUM by a rank-1 matmul against
    (1/dinv) so only a per-partition scale remains.
  - Pad slots: layer-1 pads are host-written zero rows; layer-2 stale/pad
    slots are killed by dstrow=999 (message buffers memset once + rotated so
    stale bits stay finite); table pad rows compute to exact zeros because
    their dinv entries are 0.
"""

import numpy as np
import ml_dtypes

import concourse.bacc as bacc
import concourse.bass as bass
import concourse.mybir as mybir
import concourse.tile as tile
from concourse.bass_utils import run_bass_kernel_spmd

P = 128
N_CORES = 8
BUCKETS = 4
NB_M1 = 4
NB_M2 = 8
NB_T2R = 4


def _quarter_bounds(tiles):
    """Split tiles into 4 segments; segment q's table holds all 8 cores'
    rows for those tiles (one int16 gather bucket per segment).  The first
    segment is deliberately small so its AllGather -- which gates the start
    of the layer-2 gather phase -- fires early in layer 1."""
    q0 = max(1, tiles // 6)
    rest = tiles - q0
    qt = [q0] + [(rest + 2 - i) // 3 for i in range(3)]
    qb = [0, qt[0], qt[0] + qt[1], qt[0] + qt[1] + qt[2]]
    return qb, qt

F32 = mybir.dt.float32
BF16 = mybir.dt.bfloat16
I16 = mybir.dt.int16
I32 = mybir.dt.int32
BFNP = ml_dtypes.bfloat16


def _prep(edge_index, n, n_cores):
    """Host-side graph preprocessing.

    Returns (C1, Cb2, per_core list of dicts, lpos, dinv).  lpos[v] is the
    node's local slot (t*P + p) on its core; the layer-2 halo table is laid
    out [core0 half0 | ... | core7 half0 | core0 half1 | ... | core7 half1]
    so each AllGather half is one contiguous collective.
    """
    src = np.ascontiguousarray(edge_index[0]).astype(np.int64)
    dst = np.ascontiguousarray(edge_index[1]).astype(np.int64)

    deg = (np.bincount(dst, minlength=n) + 1).astype(np.float32)  # +self-loop
    dinv = (1.0 / np.sqrt(deg)).astype(np.float32)

    shard = n // n_cores
    tiles = (shard + P - 1) // P
    last_rows = shard - (tiles - 1) * P
    qb, qt = _quarter_bounds(tiles)
    assert n_cores * max(qt) * P < (1 << 15), "quarter exceeds int16 range"

    caps = np.full(tiles, P, dtype=np.int64)
    caps[-1] = last_rows

    core_of_dst = dst // shard

    # Greedy per-core node->tile assignment balancing per-tile total
    # in-degree (keeps both the layer-1 chunk count and the layer-2 bucket
    # cells near uniform).
    lpos = np.empty(n, dtype=np.int64)
    BIG = 1 << 40
    for r in range(n_cores):
        sel = core_of_dst == r
        d_loc = (dst[sel] - r * shard).astype(np.int64)
        cnt = np.bincount(d_loc, minlength=shard)
        order = np.argsort(-cnt, kind="stable")
        tiletot = np.zeros(tiles, dtype=np.int64)
        fill = np.zeros(tiles, dtype=np.int64)
        pos = np.empty(shard, dtype=np.int64)
        for v in order:
            tt = np.where(fill >= caps, BIG, tiletot)
            t = int(np.argmin(tt))
            tiletot[t] += cnt[v]
            pos[v] = t * P + fill[t]
            fill[t] += 1
        lpos[r * shard:(r + 1) * shard] = pos

    r_of = np.arange(n) // shard
    t_of = lpos // P
    q_edges = np.array(qb + [tiles], dtype=np.int64)
    bucket_of = np.searchsorted(q_edges, t_of, side="right") - 1
    qb_a = np.array(qb, dtype=np.int64)
    qt_a = np.array(qt, dtype=np.int64)
    idx_in_bucket = (r_of * qt_a[bucket_of] * P
                     + (lpos - qb_a[bucket_of] * P))

    # Pass 1: global chunk counts so the SPMD program is uniform.
    C1e, Cb2 = 1, 1
    edata = []
    for r in range(n_cores):
        sel = core_of_dst == r
        s_r = src[sel]
        l_d = lpos[dst[sel]]
        t_e = l_d // P
        row_e = l_d % P
        grp = t_e * BUCKETS + bucket_of[s_r]
        tcnt = np.bincount(t_e, minlength=tiles)
        C1e = max(C1e, int(-(-tcnt.max() // P)))
        gcnt = np.bincount(grp, minlength=tiles * BUCKETS)
        Cb2 = max(Cb2, int(-(-gcnt.max() // P)))
        edata.append((s_r, t_e, row_e, grp))
    C1 = C1e + 1            # chunk 0 holds the self-loop rows
    K2 = BUCKETS * Cb2
    nw2 = Cb2 * P // 16
    assert Cb2 * P <= 1024, f"Cb2={Cb2} exceeds dma_gather call limit"

    per_core = []
    for r in range(n_cores):
        s_r, t_e, row_e, grp = edata[r]
        nodes_r = np.arange(r * shard, (r + 1) * shard)
        l_r = lpos[nodes_r]

        # ---- layer 1: host-gather slot map [P, tiles*C1] -> src node id
        o1 = np.lexsort((s_r, t_e))
        s1o, t1o, row1o = s_r[o1], t_e[o1], row_e[o1]
        tcnt = np.bincount(t1o, minlength=tiles)
        tstart = np.concatenate([[0], np.cumsum(tcnt)])[:-1]
        j1 = np.arange(len(s1o)) - tstart[t1o]
        c1 = j1 // P + 1
        p1 = j1 % P
        assert c1.max(initial=1) < C1
        srcmap = np.full((P, tiles * C1), -1, dtype=np.int64)
        dr1 = np.full((P, tiles * C1), 999.0, dtype=np.float32)
        srcmap[p1, t1o * C1 + c1] = s1o
        dr1[p1, t1o * C1 + c1] = row1o
        srcmap[l_r % P, (l_r // P) * C1] = nodes_r
        dr1[:, ::C1] = np.arange(P, dtype=np.float32)[:, None]

        # ---- layer 2: bucketed int16 gather indices
        o2 = np.lexsort((idx_in_bucket[s_r], grp))
        s2o, grp2, row2o = s_r[o2], grp[o2], row_e[o2]
        gcnt = np.bincount(grp2, minlength=tiles * BUCKETS)
        gstart = np.concatenate([[0], np.cumsum(gcnt)])[:-1]
        j2 = np.arange(len(s2o)) - gstart[grp2]
        c2 = j2 // P
        p2 = j2 % P
        assert c2.max(initial=0) < Cb2
        fl2 = np.full((tiles * BUCKETS, Cb2 * P), -1, dtype=np.int64)
        fl2[grp2, c2 * P + p2] = idx_in_bucket[s2o]
        fl2[gcnt == 0, 0] = 0   # empty group still needs one valid index
        cnts = np.maximum(gcnt, 1).astype(np.int32)[None, :]
        dr2 = np.full((P, tiles * K2), 999.0, dtype=np.float32)
        t2e = grp2 // BUCKETS
        b2e = grp2 % BUCKETS
        dr2[p2, t2e * K2 + b2e * Cb2 + c2] = row2o

        iw = np.empty((16, tiles * BUCKETS * nw2), dtype=np.int16)
        fl16 = fl2.astype(np.int16)
        for g in range(tiles * BUCKETS):
            iw[:, g * nw2:(g + 1) * nw2] = fl16[g].reshape(nw2, 16).T
        idxw = np.tile(iw, (8, 1))

        # ---- per-slot dinv tables (0 on pad slots)
        dd = np.zeros(tiles * P, dtype=np.float32)
        rr = np.zeros(tiles * P, dtype=np.float32)
        dd[l_r] = dinv[nodes_r]
        rr[l_r] = 1.0 / dinv[nodes_r]
        dvd = np.ascontiguousarray(dd.reshape(tiles, P).T)
        dvd2 = np.ascontiguousarray((dd * dd).reshape(tiles, P).T)
        dvrT = np.ascontiguousarray(rr[None, :])

        per_core.append(dict(srcmap=srcmap, dr1=dr1, dr2=dr2, idxw=idxw,
                             dvd=dvd, dvd2=dvd2, dvrT=dvrT, cnts=cnts))
    return C1, Cb2, per_core, lpos, dinv


def build_bass(n, fin, f1, f2, n_cores, C1, Cb2, has_bias=True):
    shard = n // n_cores
    tiles = (shard + P - 1) // P
    qb, qt = _quarter_bounds(tiles)
    K2 = BUCKETS * Cb2
    nw2 = Cb2 * P // 16

    nc = bacc.Bacc(None, target_bir_lowering=False, debug=False,
                   num_swdge_queues=4)

    m1_d = nc.declare_dram_parameter("m1", [P, tiles * C1 * fin], BF16,
                                     isOutput=False)
    w1_d = nc.declare_dram_parameter("w1", [fin, f1], F32, isOutput=False)
    w2_d = nc.declare_dram_parameter("w2", [f1, f2], F32, isOutput=False)
    b1_d = nc.declare_dram_parameter("b1r", [1, f1], BF16, isOutput=False)
    b2_d = nc.declare_dram_parameter("b2r", [1, f2], BF16, isOutput=False)
    iob_d = nc.declare_dram_parameter("iob", [P, P], BF16, isOutput=False)
    idb_d = nc.declare_dram_parameter("idb", [P, P], BF16, isOutput=False)
    dr1_d = nc.declare_dram_parameter("dr1", [P, tiles * C1], BF16,
                                      isOutput=False)
    dr2_d = nc.declare_dram_parameter("dr2", [P, tiles * K2], BF16,
                                      isOutput=False)
    idx_d = nc.declare_dram_parameter("idx2", [P, tiles * BUCKETS * nw2],
                                      I16, isOutput=False)
    dvd_d = nc.declare_dram_parameter("dvd", [P, tiles], F32, isOutput=False)
    dvd2_d = nc.declare_dram_parameter("dvd2", [P, tiles], F32,
                                       isOutput=False)
    dvr_d = None
    if has_bias:
        dvr_d = nc.declare_dram_parameter("dvrT", [1, tiles * P], BF16,
                                          isOutput=False)
    cnt_d = nc.declare_dram_parameter("cnts", [1, tiles * BUCKETS], I32,
                                      isOutput=False)
    out_d = nc.declare_dram_parameter("out", [tiles * P, f2], F32,
                                      isOutput=True)

    with tile.TileContext(nc) as tc:
        with (
            tc.tile_pool(name="dram", bufs=1, space="DRAM") as dram,
            tc.tile_pool(name="const", bufs=1) as const,
            tc.tile_pool(name="mbuf", bufs=1) as mbuf,
            tc.tile_pool(name="smat", bufs=4) as smatp,
            tc.tile_pool(name="selfp", bufs=3) as selfp,
            tc.tile_pool(name="small", bufs=4) as small,
            tc.tile_pool(name="psum_agg", bufs=4, space="PSUM") as psag,
            tc.tile_pool(name="psum_out", bufs=2, space="PSUM") as psout,
        ):
            t2_shq = [dram.tile([qt[q] * P, P], BF16, name=f"t2sh{q}")
                      for q in range(4)]
            t2f = [dram.tile([n_cores * qt[q] * P, P], BF16,
                             addr_space="Shared", name=f"t2f{q}")
                   for q in range(4)]

            def load(shape, dt, src_ap, name):
                t = const.tile(shape, dt, name=name)
                nc.sync.dma_start(out=t[:, :], in_=src_ap)
                return t

            w1_sb = load([fin, f1], F32, w1_d[:, :], "w1sb")
            w2_sb = load([f1, f2], F32, w2_d[:, :], "w2sb")
            b1_sb = load([1, f1], BF16, b1_d[:, :], "b1sb")
            b2_sb = load([1, f2], BF16, b2_d[:, :], "b2sb")
            iob_sb = load([P, P], BF16, iob_d[:, :], "iobsb")
            idb_sb = load([P, P], BF16, idb_d[:, :], "idbsb")
            dr1_sb = load([P, tiles * C1], BF16, dr1_d[:, :], "dr1sb")
            dr2_sb = load([P, tiles * K2], BF16, dr2_d[:, :], "dr2sb")
            idx_sb = load([P, tiles * BUCKETS * nw2], I16, idx_d[:, :],
                          "idxsb")
            dvd_sb = load([P, tiles], F32, dvd_d[:, :], "dvdsb")
            dvd2_sb = load([P, tiles], F32, dvd2_d[:, :], "dvd2sb")
            dvr_sb = (load([1, tiles * P], BF16, dvr_d[:, :], "dvrsb")
                      if has_bias else None)
            cnt_sb = load([1, tiles * BUCKETS], I32, cnt_d[:, :], "cntsb")
            cnt_regs = [nc.alloc_register(mybir.EngineType.Pool, f"cnt{i}")
                        for i in range(BUCKETS)]

            m1bufs = [mbuf.tile([P, C1 * fin], BF16, name=f"m1b{i}")
                      for i in range(NB_M1)]
            m2bufs = [mbuf.tile([P, Cb2 * P], BF16, name=f"m2b{i}")
                      for i in range(NB_M2)]
            t2rbufs = [mbuf.tile([P, P], BF16, name=f"t2r{i}")
                       for i in range(NB_T2R)]
            for b in m2bufs + t2rbufs:
                nc.vector.memset(b[:, :], 0.0)

            def build_S(drow_sb, col0, K, name):
                s_t = smatp.tile([P, K * P], BF16, name=name, tag="s")
                s3 = s_t[:, :].rearrange("p (k r) -> p k r", r=P)
                dm = drow_sb[:, col0:col0 + K]
                dm3 = bass.AP(dm.tensor, dm.offset, [*dm.ap, [0, P]])
                io = iob_sb[:, :]
                io3 = bass.AP(io.tensor, io.offset,
                              [io.ap[0], [0, K], io.ap[1]])
                nc.vector.tensor_tensor(out=s3, in0=dm3, in1=io3,
                                        op=mybir.AluOpType.is_equal)
                return s_t

            # Emission helpers ------------------------------------------
            swdge_issue = [0]
            gather_insts = []

            def l1_tile(t):
                msg = m1bufs[t % NB_M1]
                nc.sync.dma_start(out=msg[:, :],
                                  in_=m1_d[:, t * C1 * fin:(t + 1) * C1 * fin])
                s1 = build_S(dr1_sb, t * C1, C1, f"s1_{t}")
                agg = psag.tile([fin, P], F32, name=f"agg1_{t}", tag="agg")
                for k in range(C1):
                    nc.tensor.matmul(agg[:, :],
                                     msg[:, k * fin:(k + 1) * fin],
                                     s1[:, k * P:(k + 1) * P],
                                     start=(k == 0), stop=(k == C1 - 1))
                agg_sb = small.tile([fin, P], F32, name=f"as1_{t}",
                                    tag="aggsb")
                nc.scalar.copy(out=agg_sb[:, :], in_=agg[:, :])
                h = psout.tile([P, f1], F32, name=f"h1_{t}", tag="h")
                nc.tensor.matmul(h[:, :], agg_sb[:, :], w1_sb[:, :],
                                 start=True, stop=not has_bias)
                if has_bias:
                    nc.tensor.matmul(h[:, :],
                                     dvr_sb[0:1, t * P:(t + 1) * P],
                                     b1_sb[0:1, :], start=False, stop=True)
                t2r = t2rbufs[t % NB_T2R]
                nc.scalar.activation(out=t2r[:, 0:f1], in_=h[:, :],
                                     func=mybir.ActivationFunctionType.Relu,
                                     scale=dvd2_sb[:, t:t + 1])
                q = next(i for i in range(4)
                         if qb[i] <= t < qb[i] + qt[i])
                tt = t - qb[q]
                nc.sync.dma_start(out=t2_shq[q][tt * P:(tt + 1) * P, :],
                                  in_=t2r[:, :])
                # fire quarter q's halo AllGather as soon as its last tile
                # is stored; later quarters transfer under earlier gathers
                if t == qb[q] + qt[q] - 1:
                    nc.gpsimd.collective_compute(
                        "AllGather", mybir.AluOpType.bypass,
                        replica_groups=[list(range(n_cores))],
                        ins=[t2_shq[q][:, :].opt()],
                        outs=[t2f[q][:, :].opt()])

            def gather2(buf, t, b):
                g = t * BUCKETS + b
                reg = cnt_regs[swdge_issue[0] % 4]
                swdge_issue[0] += 1
                nc.gpsimd.reg_load(reg, cnt_sb[0:1, g:g + 1])
                gi = nc.gpsimd.dma_gather(
                    out_ap=buf[:, :].rearrange("p (c e) -> p c e", e=P),
                    in_ap=t2f[b][:, :],
                    idxs_ap=idx_sb[:, g * nw2:(g + 1) * nw2],
                    num_idxs=Cb2 * P,
                    num_idxs_reg=reg,
                    elem_size=P,
                    queue_num=0,
                )
                gather_insts.append(gi)

            aggA = mbuf.tile([f1, tiles * P], F32, name="aggA")

            def l2_tile(b, t):
                msg = m2bufs[(b * tiles + t) % NB_M2]
                gather2(msg, t, b)
                s2 = build_S(dr2_sb, t * K2 + b * Cb2, Cb2, f"s2_{b}_{t}")
                ps = psag.tile([f1, P], F32, name=f"ps_{b}_{t}", tag="agg")
                if b == 0:
                    q = next(i for i in range(4)
                             if qb[i] <= t < qb[i] + qt[i])
                    tt = t - qb[q]
                    ts2 = selfp.tile([P, P], BF16, name=f"ts2_{t}",
                                     tag="ts2")
                    nc.scalar.dma_start(
                        out=ts2[:, :],
                        in_=t2_shq[q][tt * P:(tt + 1) * P, :])
                    nc.tensor.matmul(ps[:, :], ts2[:, 0:f1], idb_sb[:, :],
                                     start=True, stop=False)
                for c in range(Cb2):
                    nc.tensor.matmul(ps[:, :], msg[:, c * P:c * P + f1],
                                     s2[:, c * P:(c + 1) * P],
                                     start=(b != 0 and c == 0),
                                     stop=(c == Cb2 - 1))
                cols = aggA[:, t * P:(t + 1) * P]
                if b == 0:
                    nc.scalar.copy(out=cols, in_=ps[:, :])
                else:
                    nc.vector.tensor_tensor(out=cols, in0=cols,
                                            in1=ps[:, :],
                                            op=mybir.AluOpType.add)
                if b == 3:
                    o = psout.tile([P, f2], F32, name=f"o_{t}", tag="h")
                    nc.tensor.matmul(o[:, :], aggA[:, t * P:(t + 1) * P],
                                     w2_sb[:, :], start=True,
                                     stop=not has_bias)
                    if has_bias:
                        nc.tensor.matmul(o[:, :],
                                         dvr_sb[0:1, t * P:(t + 1) * P],
                                         b2_sb[0:1, :],
                                         start=False, stop=True)
                    u = small.tile([P, f2], F32, name=f"u_{t}", tag="u")
                    nc.scalar.activation(
                        out=u[:, :], in_=o[:, :],
                        func=mybir.ActivationFunctionType.Copy,
                        scale=dvd_sb[:, t:t + 1])
                    nc.sync.dma_start(out=out_d[t * P:(t + 1) * P, :],
                                      in_=u[:, :])

            # Software-pipelined emission: the per-engine instruction
            # streams execute in program order, so bucket-0's layer-2 work
            # is interleaved into the tail of layer 1 -- otherwise the Q7
            # gathers throttle on message buffers whose consumers sit
            # behind all of layer 1 in the Tensor/Vector queues.
            for t in range(qb[1]):
                l1_tile(t)
            rem = tiles - qb[1]
            b0_done = 0
            for i, t in enumerate(range(qb[1], tiles)):
                l1_tile(t)
                want = (i + 1) * tiles // rem
                while b0_done < want:
                    l2_tile(0, b0_done)
                    b0_done += 1
            while b0_done < tiles:
                l2_tile(0, b0_done)
                b0_done += 1
            for b in range(1, 4):
                for t in range(tiles):
                    l2_tile(b, t)

    # Spread gathers over the 4 SWDGE queues (= 4 Q7 core pairs) so
    # descriptor generation runs 4-wide.  queue = assigned DMASW lane % 4
    # keeps every DMA-completion semaphore on a single queue, which the
    # SWDGE ring bookkeeping requires.
    import re
    for gi in gather_insts:
        u0 = str(gi.ins.sync_info.on_update[0])
        m = re.search(r"DMASW(\d+)_", u0)
        assert m, f"gather without DMASW lane sem: {u0}"
        gi.ins.queue_num = int(m.group(1)) % 4

    nc.compile()
    return nc


def make_in_maps(x, W1, b1, W2, b2, C1, Cb2, per_core, dinv,
                 has_bias=True):
    n, fin = x.shape
    f1 = W1.shape[1]
    f2 = W2.shape[1]
    shard = n // N_CORES
    tiles = (shard + P - 1) // P
    xsc = (np.asarray(x, np.float32) * dinv[:, None]).astype(BFNP)
    xsc_pad = np.concatenate([xsc, np.zeros((1, fin), dtype=BFNP)])
    iota = np.broadcast_to(np.arange(P, dtype=np.float32), (P, P))
    ident = np.eye(P, dtype=np.float32)
    w1 = np.ascontiguousarray(W1, dtype=np.float32)
    w2 = np.ascontiguousarray(W2, dtype=np.float32)
    b1r = np.asarray(b1, np.float32).reshape(1, f1).astype(BFNP)
    b2r = np.asarray(b2, np.float32).reshape(1, f2).astype(BFNP)
    in_maps = []
    for pc in per_core:
        m1 = xsc_pad[pc["srcmap"]]          # -1 wraps to the zero row
        m1 = np.ascontiguousarray(m1.reshape(P, tiles * C1 * fin))
        in_maps.append({
            "m1": m1,
            "w1": w1,
            "w2": w2,
            "b1r": b1r,
            "b2r": b2r,
            "iob": iota.astype(BFNP),
            "idb": ident.astype(BFNP),
            "dr1": pc["dr1"].astype(BFNP),
            "dr2": pc["dr2"].astype(BFNP),
            "idx2": pc["idxw"],
            "dvd": pc["dvd"],
            "dvd2": pc["dvd2"],
            "cnts": pc["cnts"],
        })
        if has_bias:
            in_maps[-1]["dvrT"] = pc["dvrT"].astype(BFNP)
    return in_maps


def kernel(x, edge_index, W1, b1, W2, b2, _trace=False):
    n, fin = x.shape
    f1 = W1.shape[1]
    f2 = W2.shape[1]
    shard = n // N_CORES

    has_bias = bool(np.any(np.asarray(b1)) or np.any(np.asarray(b2)))
    C1, Cb2, per_core, lpos, dinv = _prep(np.asarray(edge_index), n, N_CORES)
    nc = build_bass(n, fin, f1, f2, N_CORES, C1, Cb2, has_bias=has_bias)
    in_maps = make_in_maps(x, W1, b1, W2, b2, C1, Cb2, per_core, dinv,
                           has_bias=has_bias)
    res = run_bass_kernel_spmd(nc, in_maps, core_ids=list(range(N_CORES)),
                               trace=_trace)
    dev = np.stack([np.asarray(res.results[r]["out"], dtype=np.float32)
                    for r in range(N_CORES)])
    core_of = np.arange(n) // shard
    full = dev[core_of, lpos]
    if _trace:
        kernel.last_exec_time_ns = res.exec_time_ns
        kernel.last_results = res
    return full


# revision 29
# speedup vs baseline: 1.0536x; 1.0536x over previous
"""Two-layer GCN (PyG GCNConv x2 + ReLU) on 8 Trainium2 NeuronCores.

Strategy (graph/data parallel, dst-partitioned):
  - Nodes are sharded across the 8 cores (12500 each); edges are partitioned
    by destination node so every scatter-add is core-local, accumulated in
    PSUM per 128-row output tile via matmuls against a selection matrix
    S[edge, row] = (dstrow[edge] == row) built with one is_equal per tile.
  - All normalization is folded into the data: layer-1 streams host
    pre-gathered, dinv-pre-scaled source rows (bf16) per tile -- a pure
    contiguous HWDGE DMA, no descriptor generation on the Q7.  The per-tile
    self-loop rows are chunk 0 of the same stream with dstrow = iota.
  - Layer-2 messages are gathered on-device from an AllGathered table of
    dinv-pre-scaled layer-1 outputs (bf16 rows padded to 256 B).  The 392
    InstDMAGatherAnt calls are striped across all 4 SWDGE queues so all 8
    Q7 cores generate descriptors concurrently (the ucode dedicates core
    pair 2q/2q+1 to queue q; a single queue leaves 6 cores idle).
  - The halo AllGather is split in two halves (table laid out as all cores'
    first half-shards then all second half-shards) and the bucket-2/3
    gathers are issued GATHER_DELAY tiles behind the bucket-0/1 gathers, so
    the second collective transfers underneath early gathers.
  - Per-node scales ride the Scalar (ACT) engine: t2 = relu(dinv^2*(agg@W1)
    + dinv*b1) and out = dinv*(agg@W2) + b2 are one activation instruction
    each, with the bias term added in PSUM by a rank-1 matmul against
    (1/dinv) so only a per-partition scale remains.
  - Pad slots: layer-1 pads are host-written zero rows; layer-2 stale/pad
    slots are killed by dstrow=999 (message buffers memset once + rotated so
    stale bits stay finite); table pad rows compute to exact zeros because
    their dinv entries are 0.
"""

import numpy as np
import ml_dtypes

import concourse.bacc as bacc
import concourse.bass as bass
import concourse.mybir as mybir
import concourse.tile as tile
from concourse.bass_utils import run_bass_kernel_spmd

P = 128
N_CORES = 8
BUCKETS = 4
NB_M1 = 4
NB_M2 = 8
NB_T2R = 4


def _quarter_bounds(tiles):
    """Split tiles into 4 segments; segment q's table holds all 8 cores'
    rows for those tiles (one int16 gather bucket per segment).  The first
    segment is deliberately small so its AllGather -- which gates the start
    of the layer-2 gather phase -- fires early in layer 1."""
    q0 = max(1, tiles // 12)
    rest = tiles - q0
    qt = [q0] + [(rest + 2 - i) // 3 for i in range(3)]
    qb = [0, qt[0], qt[0] + qt[1], qt[0] + qt[1] + qt[2]]
    return qb, qt

F32 = mybir.dt.float32
BF16 = mybir.dt.bfloat16
I16 = mybir.dt.int16
I32 = mybir.dt.int32
BFNP = ml_dtypes.bfloat16


def _prep(edge_index, n, n_cores):
    """Host-side graph preprocessing.

    Returns (C1, Cb2, per_core list of dicts, lpos, dinv).  lpos[v] is the
    node's local slot (t*P + p) on its core; the layer-2 halo table is laid
    out [core0 half0 | ... | core7 half0 | core0 half1 | ... | core7 half1]
    so each AllGather half is one contiguous collective.
    """
    src = np.ascontiguousarray(edge_index[0]).astype(np.int64)
    dst = np.ascontiguousarray(edge_index[1]).astype(np.int64)

    deg = (np.bincount(dst, minlength=n) + 1).astype(np.float32)  # +self-loop
    dinv = (1.0 / np.sqrt(deg)).astype(np.float32)

    shard = n // n_cores
    tiles = (shard + P - 1) // P
    last_rows = shard - (tiles - 1) * P
    qb, qt = _quarter_bounds(tiles)
    assert n_cores * max(qt) * P < (1 << 15), "quarter exceeds int16 range"

    caps = np.full(tiles, P, dtype=np.int64)
    caps[-1] = last_rows

    core_of_dst = dst // shard

    # Greedy per-core node->tile assignment balancing per-tile total
    # in-degree (keeps both the layer-1 chunk count and the layer-2 bucket
    # cells near uniform).
    lpos = np.empty(n, dtype=np.int64)
    BIG = 1 << 40
    for r in range(n_cores):
        sel = core_of_dst == r
        d_loc = (dst[sel] - r * shard).astype(np.int64)
        cnt = np.bincount(d_loc, minlength=shard)
        order = np.argsort(-cnt, kind="stable")
        tiletot = np.zeros(tiles, dtype=np.int64)
        fill = np.zeros(tiles, dtype=np.int64)
        pos = np.empty(shard, dtype=np.int64)
        for v in order:
            tt = np.where(fill >= caps, BIG, tiletot)
            t = int(np.argmin(tt))
            tiletot[t] += cnt[v]
            pos[v] = t * P + fill[t]
            fill[t] += 1
        lpos[r * shard:(r + 1) * shard] = pos

    r_of = np.arange(n) // shard
    t_of = lpos // P
    q_edges = np.array(qb + [tiles], dtype=np.int64)
    bucket_of = np.searchsorted(q_edges, t_of, side="right") - 1
    qb_a = np.array(qb, dtype=np.int64)
    qt_a = np.array(qt, dtype=np.int64)
    idx_in_bucket = (r_of * qt_a[bucket_of] * P
                     + (lpos - qb_a[bucket_of] * P))

    # Pass 1: global chunk counts so the SPMD program is uniform.  Chunk
    # counts are per BUCKET (a small first quarter means small bucket-0
    # cells), sized by the max cell over all cores and tiles.
    C1e = 1
    CBm = np.ones(BUCKETS, dtype=np.int64)
    edata = []
    for r in range(n_cores):
        sel = core_of_dst == r
        s_r = src[sel]
        l_d = lpos[dst[sel]]
        t_e = l_d // P
        row_e = l_d % P
        grp = t_e * BUCKETS + bucket_of[s_r]
        tcnt = np.bincount(t_e, minlength=tiles)
        C1e = max(C1e, int(-(-tcnt.max() // P)))
        gcnt = np.bincount(grp, minlength=tiles * BUCKETS)
        gmax = gcnt.reshape(tiles, BUCKETS).max(axis=0)
        CBm = np.maximum(CBm, -(-gmax // P))
    C1 = C1e + 1            # chunk 0 holds the self-loop rows
    CB = [int(c) for c in CBm]
    KS = sum(CB)
    cb_off = np.concatenate([[0], np.cumsum(CB)])[:-1]
    nw_b = [c * P // 16 for c in CB]
    nwS = sum(nw_b)
    nw_off = np.concatenate([[0], np.cumsum(nw_b)])[:-1]
    for c in CB:
        assert c * P <= 1024, f"C_b={c} exceeds dma_gather call limit"

    per_core = []
    for r in range(n_cores):
        sel = core_of_dst == r
        s_r = src[sel]
        l_d = lpos[dst[sel]]
        t_e = l_d // P
        row_e = l_d % P
        grp = t_e * BUCKETS + bucket_of[s_r]
        nodes_r = np.arange(r * shard, (r + 1) * shard)
        l_r = lpos[nodes_r]

        # ---- layer 1: host-gather slot map [P, tiles*C1] -> src node id
        o1 = np.lexsort((s_r, t_e))
        s1o, t1o, row1o = s_r[o1], t_e[o1], row_e[o1]
        tcnt = np.bincount(t1o, minlength=tiles)
        tstart = np.concatenate([[0], np.cumsum(tcnt)])[:-1]
        j1 = np.arange(len(s1o)) - tstart[t1o]
        c1 = j1 // P + 1
        p1 = j1 % P
        assert c1.max(initial=1) < C1
        srcmap = np.full((P, tiles * C1), -1, dtype=np.int64)
        dr1 = np.full((P, tiles * C1), 999.0, dtype=np.float32)
        srcmap[p1, t1o * C1 + c1] = s1o
        dr1[p1, t1o * C1 + c1] = row1o
        srcmap[l_r % P, (l_r // P) * C1] = nodes_r
        dr1[:, ::C1] = np.arange(P, dtype=np.float32)[:, None]

        # ---- layer 2: bucketed int16 gather indices (per-bucket widths)
        o2 = np.lexsort((idx_in_bucket[s_r], grp))
        s2o, grp2, row2o = s_r[o2], grp[o2], row_e[o2]
        gcnt = np.bincount(grp2, minlength=tiles * BUCKETS)
        gstart = np.concatenate([[0], np.cumsum(gcnt)])[:-1]
        j2 = np.arange(len(s2o)) - gstart[grp2]
        c2 = j2 // P
        p2 = j2 % P
        b_e2 = grp2 % BUCKETS
        assert np.all(c2 < CBm[b_e2])
        cnts = np.maximum(gcnt, 1).astype(np.int32)[None, :]
        dr2 = np.full((P, tiles * KS), 999.0, dtype=np.float32)
        t2e = grp2 // BUCKETS
        dr2[p2, t2e * KS + cb_off[b_e2] + c2] = row2o

        flat = np.full(tiles * KS * P, -1, dtype=np.int16)
        flat[(t2e * KS + cb_off[b_e2] + c2) * P + p2] = (
            idx_in_bucket[s2o].astype(np.int16))
        empty = np.nonzero(gcnt == 0)[0]
        if len(empty):
            eb = empty % BUCKETS
            flat[(empty // BUCKETS * KS + cb_off[eb]) * P] = 0
        iw = np.empty((16, tiles, nwS), dtype=np.int16)
        flat3 = flat.reshape(tiles, KS * P)
        for b in range(BUCKETS):
            fb = flat3[:, cb_off[b] * P:(cb_off[b] + CB[b]) * P]
            iw[:, :, nw_off[b]:nw_off[b] + nw_b[b]] = (
                fb.reshape(tiles, nw_b[b], 16).transpose(2, 0, 1))
        idxw = np.tile(iw.reshape(16, tiles * nwS), (8, 1))

        # ---- per-slot dinv tables (0 on pad slots)
        dd = np.zeros(tiles * P, dtype=np.float32)
        rr = np.zeros(tiles * P, dtype=np.float32)
        dd[l_r] = dinv[nodes_r]
        rr[l_r] = 1.0 / dinv[nodes_r]
        dvd = np.ascontiguousarray(dd.reshape(tiles, P).T)
        dvd2 = np.ascontiguousarray((dd * dd).reshape(tiles, P).T)
        dvrT = np.ascontiguousarray(rr[None, :])

        per_core.append(dict(srcmap=srcmap, dr1=dr1, dr2=dr2, idxw=idxw,
                             dvd=dvd, dvd2=dvd2, dvrT=dvrT, cnts=cnts))
    return C1, CB, per_core, lpos, dinv


def build_bass(n, fin, f1, f2, n_cores, C1, CB, has_bias=True):
    shard = n // n_cores
    tiles = (shard + P - 1) // P
    qb, qt = _quarter_bounds(tiles)
    KS = sum(CB)
    Cmax = max(CB)
    cb_off = [0, CB[0], CB[0] + CB[1], CB[0] + CB[1] + CB[2]]
    nw_b = [c * P // 16 for c in CB]
    nwS = sum(nw_b)
    nw_off = [0, nw_b[0], nw_b[0] + nw_b[1], nw_b[0] + nw_b[1] + nw_b[2]]

    nc = bacc.Bacc(None, target_bir_lowering=False, debug=False,
                   num_swdge_queues=4)

    m1_d = nc.declare_dram_parameter("m1", [P, tiles * C1 * fin], BF16,
                                     isOutput=False)
    w1_d = nc.declare_dram_parameter("w1", [fin, f1], F32, isOutput=False)
    w2_d = nc.declare_dram_parameter("w2", [f1, f2], F32, isOutput=False)
    b1_d = nc.declare_dram_parameter("b1r", [1, f1], BF16, isOutput=False)
    b2_d = nc.declare_dram_parameter("b2r", [1, f2], BF16, isOutput=False)
    iob_d = nc.declare_dram_parameter("iob", [P, P], BF16, isOutput=False)
    idb_d = nc.declare_dram_parameter("idb", [P, P], BF16, isOutput=False)
    dr1_d = nc.declare_dram_parameter("dr1", [P, tiles * C1], BF16,
                                      isOutput=False)
    dr2_d = nc.declare_dram_parameter("dr2", [P, tiles * KS], BF16,
                                      isOutput=False)
    idx_d = nc.declare_dram_parameter("idx2", [P, tiles * nwS],
                                      I16, isOutput=False)
    dvd_d = nc.declare_dram_parameter("dvd", [P, tiles], F32, isOutput=False)
    dvd2_d = nc.declare_dram_parameter("dvd2", [P, tiles], F32,
                                       isOutput=False)
    dvr_d = None
    if has_bias:
        dvr_d = nc.declare_dram_parameter("dvrT", [1, tiles * P], BF16,
                                          isOutput=False)
    cnt_d = nc.declare_dram_parameter("cnts", [1, tiles * BUCKETS], I32,
                                      isOutput=False)
    out_d = nc.declare_dram_parameter("out", [tiles * P, f2], F32,
                                      isOutput=True)

    with tile.TileContext(nc) as tc:
        with (
            tc.tile_pool(name="dram", bufs=1, space="DRAM") as dram,
            tc.tile_pool(name="const", bufs=1) as const,
            tc.tile_pool(name="mbuf", bufs=1) as mbuf,
            tc.tile_pool(name="smat", bufs=4) as smatp,
            tc.tile_pool(name="selfp", bufs=3) as selfp,
            tc.tile_pool(name="small", bufs=4) as small,
            tc.tile_pool(name="psum_agg", bufs=4, space="PSUM") as psag,
            tc.tile_pool(name="psum_out", bufs=2, space="PSUM") as psout,
        ):
            t2_shq = [dram.tile([qt[q] * P, P], BF16, name=f"t2sh{q}")
                      for q in range(4)]
            t2f = [dram.tile([n_cores * qt[q] * P, P], BF16,
                             addr_space="Shared", name=f"t2f{q}")
                   for q in range(4)]

            def load(shape, dt, src_ap, name):
                t = const.tile(shape, dt, name=name)
                nc.sync.dma_start(out=t[:, :], in_=src_ap)
                return t

            w1_sb = load([fin, f1], F32, w1_d[:, :], "w1sb")
            w2_sb = load([f1, f2], F32, w2_d[:, :], "w2sb")
            b1_sb = load([1, f1], BF16, b1_d[:, :], "b1sb")
            b2_sb = load([1, f2], BF16, b2_d[:, :], "b2sb")
            iob_sb = load([P, P], BF16, iob_d[:, :], "iobsb")
            idb_sb = load([P, P], BF16, idb_d[:, :], "idbsb")
            dr1_sb = load([P, tiles * C1], BF16, dr1_d[:, :], "dr1sb")
            dr2_sb = load([P, tiles * KS], BF16, dr2_d[:, :], "dr2sb")
            idx_sb = load([P, tiles * nwS], I16, idx_d[:, :], "idxsb")
            dvd_sb = load([P, tiles], F32, dvd_d[:, :], "dvdsb")
            dvd2_sb = load([P, tiles], F32, dvd2_d[:, :], "dvd2sb")
            dvr_sb = (load([1, tiles * P], BF16, dvr_d[:, :], "dvrsb")
                      if has_bias else None)
            cnt_sb = load([1, tiles * BUCKETS], I32, cnt_d[:, :], "cntsb")
            cnt_regs = [nc.alloc_register(mybir.EngineType.Pool, f"cnt{i}")
                        for i in range(BUCKETS)]

            m1bufs = [mbuf.tile([P, C1 * fin], BF16, name=f"m1b{i}")
                      for i in range(NB_M1)]
            m2bufs = [mbuf.tile([P, Cmax * P], BF16, name=f"m2b{i}")
                      for i in range(NB_M2)]
            t2rbufs = [mbuf.tile([P, P], BF16, name=f"t2r{i}")
                       for i in range(NB_T2R)]
            for b in m2bufs + t2rbufs:
                nc.vector.memset(b[:, :], 0.0)

            def build_S(drow_sb, col0, K, name):
                s_t = smatp.tile([P, K * P], BF16, name=name, tag="s")
                s3 = s_t[:, :].rearrange("p (k r) -> p k r", r=P)
                dm = drow_sb[:, col0:col0 + K]
                dm3 = bass.AP(dm.tensor, dm.offset, [*dm.ap, [0, P]])
                io = iob_sb[:, :]
                io3 = bass.AP(io.tensor, io.offset,
                              [io.ap[0], [0, K], io.ap[1]])
                nc.vector.tensor_tensor(out=s3, in0=dm3, in1=io3,
                                        op=mybir.AluOpType.is_equal)
                return s_t

            # Emission helpers ------------------------------------------
            swdge_issue = [0]
            gather_insts = []

            def l1_tile(t):
                msg = m1bufs[t % NB_M1]
                nc.sync.dma_start(out=msg[:, :],
                                  in_=m1_d[:, t * C1 * fin:(t + 1) * C1 * fin])
                s1 = build_S(dr1_sb, t * C1, C1, f"s1_{t}")
                agg = psag.tile([fin, P], F32, name=f"agg1_{t}", tag="agg")
                for k in range(C1):
                    nc.tensor.matmul(agg[:, :],
                                     msg[:, k * fin:(k + 1) * fin],
                                     s1[:, k * P:(k + 1) * P],
                                     start=(k == 0), stop=(k == C1 - 1))
                agg_sb = small.tile([fin, P], F32, name=f"as1_{t}",
                                    tag="aggsb")
                nc.scalar.copy(out=agg_sb[:, :], in_=agg[:, :])
                h = psout.tile([P, f1], F32, name=f"h1_{t}", tag="h")
                nc.tensor.matmul(h[:, :], agg_sb[:, :], w1_sb[:, :],
                                 start=True, stop=not has_bias)
                if has_bias:
                    nc.tensor.matmul(h[:, :],
                                     dvr_sb[0:1, t * P:(t + 1) * P],
                                     b1_sb[0:1, :], start=False, stop=True)
                t2r = t2rbufs[t % NB_T2R]
                nc.scalar.activation(out=t2r[:, 0:f1], in_=h[:, :],
                                     func=mybir.ActivationFunctionType.Relu,
                                     scale=dvd2_sb[:, t:t + 1])
                q = next(i for i in range(4)
                         if qb[i] <= t < qb[i] + qt[i])
                tt = t - qb[q]
                nc.sync.dma_start(out=t2_shq[q][tt * P:(tt + 1) * P, :],
                                  in_=t2r[:, :])
                # fire quarter q's halo AllGather as soon as its last tile
                # is stored; later quarters transfer under earlier gathers
                if t == qb[q] + qt[q] - 1:
                    nc.gpsimd.collective_compute(
                        "AllGather", mybir.AluOpType.bypass,
                        replica_groups=[list(range(n_cores))],
                        ins=[t2_shq[q][:, :].opt()],
                        outs=[t2f[q][:, :].opt()])

            def gather2(buf, t, b):
                g = t * BUCKETS + b
                reg = cnt_regs[swdge_issue[0] % 4]
                swdge_issue[0] += 1
                nc.gpsimd.reg_load(reg, cnt_sb[0:1, g:g + 1])
                col = t * nwS + nw_off[b]
                gi = nc.gpsimd.dma_gather(
                    out_ap=buf[:, :CB[b] * P]
                    .rearrange("p (c e) -> p c e", e=P),
                    in_ap=t2f[b][:, :],
                    idxs_ap=idx_sb[:, col:col + nw_b[b]],
                    num_idxs=CB[b] * P,
                    num_idxs_reg=reg,
                    elem_size=P,
                    queue_num=0,
                )
                gather_insts.append(gi)

            aggA = mbuf.tile([f1, tiles * P], F32, name="aggA")

            def l2_tile(b, t):
                msg = m2bufs[(b * tiles + t) % NB_M2]
                gather2(msg, t, b)
                s2 = build_S(dr2_sb, t * KS + cb_off[b], CB[b],
                             f"s2_{b}_{t}")
                ps = psag.tile([f1, P], F32, name=f"ps_{b}_{t}", tag="agg")
                if b == 0:
                    q = next(i for i in range(4)
                             if qb[i] <= t < qb[i] + qt[i])
                    tt = t - qb[q]
                    ts2 = selfp.tile([P, P], BF16, name=f"ts2_{t}",
                                     tag="ts2")
                    nc.scalar.dma_start(
                        out=ts2[:, :],
                        in_=t2_shq[q][tt * P:(tt + 1) * P, :])
                    nc.tensor.matmul(ps[:, :], ts2[:, 0:f1], idb_sb[:, :],
                                     start=True, stop=False)
                for c in range(CB[b]):
                    nc.tensor.matmul(ps[:, :], msg[:, c * P:c * P + f1],
                                     s2[:, c * P:(c + 1) * P],
                                     start=(b != 0 and c == 0),
                                     stop=(c == CB[b] - 1))
                cols = aggA[:, t * P:(t + 1) * P]
                if b == 0:
                    nc.scalar.copy(out=cols, in_=ps[:, :])
                else:
                    nc.vector.tensor_tensor(out=cols, in0=cols,
                                            in1=ps[:, :],
                                            op=mybir.AluOpType.add)
                if b == 3:
                    o = psout.tile([P, f2], F32, name=f"o_{t}", tag="h")
                    nc.tensor.matmul(o[:, :], aggA[:, t * P:(t + 1) * P],
                                     w2_sb[:, :], start=True,
                                     stop=not has_bias)
                    if has_bias:
                        nc.tensor.matmul(o[:, :],
                                         dvr_sb[0:1, t * P:(t + 1) * P],
                                         b2_sb[0:1, :],
                                         start=False, stop=True)
                    u = small.tile([P, f2], F32, name=f"u_{t}", tag="u")
                    nc.scalar.activation(
                        out=u[:, :], in_=o[:, :],
                        func=mybir.ActivationFunctionType.Copy,
                        scale=dvd_sb[:, t:t + 1])
                    nc.sync.dma_start(out=out_d[t * P:(t + 1) * P, :],
                                      in_=u[:, :])

            # Software-pipelined emission: the per-engine instruction
            # streams execute in program order, so bucket-0's layer-2 work
            # is interleaved into the tail of layer 1 -- otherwise the Q7
            # gathers throttle on message buffers whose consumers sit
            # behind all of layer 1 in the Tensor/Vector queues.
            for t in range(qb[1]):
                l1_tile(t)
            rem = tiles - qb[1]
            b0_done = 0
            for i, t in enumerate(range(qb[1], tiles)):
                l1_tile(t)
                want = (i + 1) * tiles // rem
                while b0_done < want:
                    l2_tile(0, b0_done)
                    b0_done += 1
            while b0_done < tiles:
                l2_tile(0, b0_done)
                b0_done += 1
            for b in range(1, 4):
                for t in range(tiles):
                    l2_tile(b, t)

    # Spread gathers over the 4 SWDGE queues (= 4 Q7 core pairs) so
    # descriptor generation runs 4-wide.  queue = assigned DMASW lane % 4
    # keeps every DMA-completion semaphore on a single queue, which the
    # SWDGE ring bookkeeping requires.
    import re
    for gi in gather_insts:
        u0 = str(gi.ins.sync_info.on_update[0])
        m = re.search(r"DMASW(\d+)_", u0)
        assert m, f"gather without DMASW lane sem: {u0}"
        gi.ins.queue_num = int(m.group(1)) % 4

    nc.compile()
    return nc


def make_in_maps(x, W1, b1, W2, b2, C1, CB, per_core, dinv,
                 has_bias=True):
    n, fin = x.shape
    f1 = W1.shape[1]
    f2 = W2.shape[1]
    shard = n // N_CORES
    tiles = (shard + P - 1) // P
    xsc = (np.asarray(x, np.float32) * dinv[:, None]).astype(BFNP)
    xsc_pad = np.concatenate([xsc, np.zeros((1, fin), dtype=BFNP)])
    iota = np.broadcast_to(np.arange(P, dtype=np.float32), (P, P))
    ident = np.eye(P, dtype=np.float32)
    w1 = np.ascontiguousarray(W1, dtype=np.float32)
    w2 = np.ascontiguousarray(W2, dtype=np.float32)
    b1r = np.asarray(b1, np.float32).reshape(1, f1).astype(BFNP)
    b2r = np.asarray(b2, np.float32).reshape(1, f2).astype(BFNP)
    in_maps = []
    for pc in per_core:
        m1 = xsc_pad[pc["srcmap"]]          # -1 wraps to the zero row
        m1 = np.ascontiguousarray(m1.reshape(P, tiles * C1 * fin))
        in_maps.append({
            "m1": m1,
            "w1": w1,
            "w2": w2,
            "b1r": b1r,
            "b2r": b2r,
            "iob": iota.astype(BFNP),
            "idb": ident.astype(BFNP),
            "dr1": pc["dr1"].astype(BFNP),
            "dr2": pc["dr2"].astype(BFNP),
            "idx2": pc["idxw"],
            "dvd": pc["dvd"],
            "dvd2": pc["dvd2"],
            "cnts": pc["cnts"],
        })
        if has_bias:
            in_maps[-1]["dvrT"] = pc["dvrT"].astype(BFNP)
    return in_maps


def kernel(x, edge_index, W1, b1, W2, b2, _trace=False):
    n, fin = x.shape
    f1 = W1.shape[1]
    f2 = W2.shape[1]
    shard = n // N_CORES

    has_bias = bool(np.any(np.asarray(b1)) or np.any(np.asarray(b2)))
    C1, CB, per_core, lpos, dinv = _prep(np.asarray(edge_index), n, N_CORES)
    nc = build_bass(n, fin, f1, f2, N_CORES, C1, CB, has_bias=has_bias)
    in_maps = make_in_maps(x, W1, b1, W2, b2, C1, CB, per_core, dinv,
                           has_bias=has_bias)
    res = run_bass_kernel_spmd(nc, in_maps, core_ids=list(range(N_CORES)),
                               trace=_trace)
    dev = np.stack([np.asarray(res.results[r]["out"], dtype=np.float32)
                    for r in range(N_CORES)])
    core_of = np.arange(n) // shard
    full = dev[core_of, lpos]
    if _trace:
        kernel.last_exec_time_ns = res.exec_time_ns
        kernel.last_results = res
    return full
